# revision 43
# baseline (speedup 1.0000x reference)
"""LID detector kernel for Trainium2 (8 NeuronCores, data-parallel over batch).

Per core (batch shard of 32 samples):
  - features arrive host-transposed [B, HW, C] in fp8-e4m3; spatial mean
    pooling runs on the TensorEngine as ones-vector matmuls (reduction over
    the partition axis = hw), accumulating q directly in [C, B] layout in
    PSUM.  Layer 0 is pair-interleaved ([16, HW, 2*C]) so each matmul pools
    two samples.
  - reference tables arrive host-transposed [C, R] fp8; rn2 = ||r||^2 via
    square (ACT/gpsimd) + ones matmul.
  - -d2 = 2q.r - rn2 - qn2 accumulated fully in PSUM: C-chunk matmuls plus
    one K=2 matmul with lhsT [[-qn2],[-1]] and rhs [[ones],[rn2]]; eviction
    is then a plain copy into the topk buffer.
  - per-layer pipelines: each layer's pooling, qn2, and distance matmuls run
    as soon as its feature DMA lands; layer 1 is loaded last (cheapest tail).
  - top-24 smallest d2 via DVE max8 + match_replace on two column halves,
    then a 48-wide merge; LID = -2k / (sum_{i=2..21} ln d2_i - 20 ln d2_21).
  - logit = w . lid + b; sigmoid computed as 1/(1+exp(-logit)) so Ln and Exp
    share one ACT table set (preloaded by a dummy Ln at kernel start).
Sample order inside a core is PERM (evens then odds, from the layer-0 pair
packing); the host inverts it on gather.
"""

import sys

for _p in ("/opt/trn_rl_repo", "/root/.axon_site/_ro/trn_rl_repo"):
    if _p not in sys.path:
        sys.path.append(_p)

import ml_dtypes
import numpy as np

import concourse.mybir as mybir
from concourse import bass, bacc
from concourse.tile import TileContext
from concourse.bass_utils import run_bass_kernel_spmd

F32 = mybir.dt.float32
BF16 = mybir.dt.bfloat16
FP8 = mybir.dt.float8e4
NP_FP8 = ml_dtypes.float8_e4m3

N_CORES = 8
B = 32          # batch shard per core
R = 2000
K = 20
NEG_BIG = -3.0e38
LAYERS = [(64, 3136), (128, 784), (256, 196), (512, 49)]  # (C, H*W)

# column j of the on-device layout holds sample PERM[j] of the local shard
PERM = np.array([2 * j for j in range(16)] + [2 * j + 1 for j in range(16)])

# qTe column base per layer (layer 0 packed 2-per-column at 224:240)
QCOL = {3: 0, 2: 128, 1: 192, 0: 224}
# qc lhsT column base per layer within qcx [2, 128]
QCC = {3: 0, 2: 32, 1: 64, 0: 96}
# distance column halves, each split into <=512-wide matmul sub-chunks
HALves = [(0, [(0, 512), (512, 488)]), (1000, [(1000, 512), (1512, 488)])]


def build_nc():
    nc = bacc.Bacc("TRN2", target_bir_lowering=False, debug=False,
                   num_devices=N_CORES)

    feats = [nc.dram_tensor(
        "feat0" if l == 0 else f"feat{l}",
        [B // 2, HW, 2 * C] if l == 0 else [B, HW, C],
        FP8, kind="ExternalInput") for l, (C, HW) in enumerate(LAYERS)]
    rts = [nc.dram_tensor(f"rt{l}", [C, R], FP8, kind="ExternalInput")
           for l, (C, _) in enumerate(LAYERS)]
    # regwb = [-2K * w, b]  (folded on the host)
    regwb = nc.dram_tensor("regwb", [1, 5], F32, kind="ExternalInput")
    out = nc.dram_tensor("out", [B, 1], F32, kind="ExternalOutput")

    rt_chunks = {l: list(range(0, C, 128)) for l, (C, _) in enumerate(LAYERS)}

    with TileContext(nc) as tc:
        with (
            tc.tile_pool(name="pp", bufs=1) as pp,
            tc.tile_pool(name="sq", bufs=4) as sqp,
            tc.tile_pool(name="pR", bufs=1, space=bass.MemorySpace.PSUM) as pR,
            tc.tile_pool(name="pQ", bufs=1, space=bass.MemorySpace.PSUM) as pQ,
            tc.tile_pool(name="pD", bufs=3, space=bass.MemorySpace.PSUM) as pD,
        ):
            # ---------------- persistent SBUF tiles + constants
            ones8 = pp.tile([128, 1], FP8, tag="ones8", name="ones8")
            onesb = pp.tile([128, 1], BF16, tag="onesb", name="onesb")
            ones_row = pp.tile([1, B], F32, tag="ones_row", name="ones_row")
            nc.vector.memset(ones8[:], 1.0)
            nc.vector.memset(onesb[:], 1.0)
            nc.vector.memset(ones_row[:], 1.0)
            # preload the natural_log_exp_and_others ACT table set; all ACT
            # ops used below (copy/square/ln/exp) live in this one set, so no
            # further table loads are inserted
            nc.scalar.add_instruction(mybir.InstLoadActFuncSet(
                name=nc.get_next_instruction_name(), ins=[], outs=[],
                act_func_set_id=6))

            wb_sb = pp.tile([1, 5], F32, tag="wb_sb", name="wb_sb")
            nc.sync.dma_start(out=wb_sb[:], in_=regwb[:])

            rt = {}
            for l, (C, _) in enumerate(LAYERS):
                for i, c0 in enumerate(rt_chunks[l]):
                    Cc = min(128, C - c0)
                    rt[(l, i)] = pp.tile([Cc, R], FP8, tag=f"rt{l}_{i}",
                                         name=f"rt{l}_{i}")

            # combo-matmul operands (engines may only write partition 0; the
            # partition-1 rows are filled by small SBUF->SBUF DMAs):
            #   lhsT qcx: row0 = -qn2 per sample, row1 = -1
            #   rhs  rc_all: row0 = ones, row1 = rn2  (cols l*R + c)
            rn2sb = pp.tile([1, 4 * R], BF16, tag="rn2sb", name="rn2sb")
            rc_all = pp.tile([2, 4 * R], BF16, tag="rc_all", name="rc_all")
            qcx = pp.tile([2, 128], BF16, tag="qcx", name="qcx")
            negones = pp.tile([1, 128], BF16, tag="negones", name="negones")
            nc.vector.memset(rc_all[0:1, :], 1.0)
            nc.vector.memset(negones[:], -1.0)
            nc.sync.dma_start(out=qcx[1:2, :], in_=negones[:])

            # ---------------- DMA issue order = rough schedule
            for l in (0, 3, 2, 1):
                for i, c0 in enumerate(rt_chunks[l]):
                    Cc = min(128, LAYERS[l][0] - c0)
                    nc.sync.dma_start(out=rt[(l, i)][:],
                                      in_=rts[l][c0:c0 + Cc, :])

            C0, HW0 = LAYERS[0]   # 3136 = 3*1024 + 8*8; free = (pair, 8, 2C)
            f0 = [pp.tile([128, 16, 8, 2 * C0], FP8, tag=f"f0_{t}",
                          name=f"f0_{t}") for t in range(3)]
            for t in range(3):
                nc.sync.dma_start(
                    out=f0[t][:],
                    in_=bass.AP(feats[0], t * 1024 * 2 * C0,
                                [[16 * C0, 128], [HW0 * 2 * C0, 16],
                                 [1, 16 * C0]]))
            f0t = pp.tile([8, 16, 8, 2 * C0], FP8, tag="f0t", name="f0t")
            nc.sync.dma_start(
                out=f0t[:], in_=bass.AP(feats[0], 3072 * 2 * C0,
                                        [[16 * C0, 8], [HW0 * 2 * C0, 16],
                                         [1, 16 * C0]]))

            C3, HW3 = LAYERS[3]
            f3 = pp.tile([49, B, C3], FP8, tag="f3", name="f3")
            nc.sync.dma_start(
                out=f3[:], in_=bass.AP(feats[3], 0,
                                       [[C3, 49], [HW3 * C3, B], [1, C3]]))

            C2, HW2 = LAYERS[2]
            f2 = pp.tile([98, B, 2, C2], FP8, tag="f2", name="f2")
            nc.sync.dma_start(
                out=f2[:], in_=bass.AP(feats[2], 0,
                                       [[2 * C2, 98], [HW2 * C2, B],
                                        [1, 2 * C2]]))

            C1, HW1 = LAYERS[1]   # 784 = 128*4 + 68*4
            f1a = pp.tile([128, B, 4, C1], FP8, tag="f1a", name="f1a")
            nc.sync.dma_start(
                out=f1a[:], in_=bass.AP(feats[1], 0,
                                        [[4 * C1, 128], [HW1 * C1, B],
                                         [1, 4 * C1]]))
            f1b = pp.tile([68, B, 4, C1], FP8, tag="f1b", name="f1b")
            nc.sync.dma_start(
                out=f1b[:], in_=bass.AP(feats[1], 512 * C1,
                                        [[4 * C1, 68], [HW1 * C1, B],
                                         [1, 4 * C1]]))

            # ---------------- rn2 = sum_c r^2 (order: l0, l3 on ACT;
            # l2, l1 on gpsimd)
            for l, sq_eng in ((0, "act"), (3, "act"), (2, "gps"), (1, "gps")):
                chunks = rt_chunks[l]
                sqs = []
                for i, c0 in enumerate(chunks):
                    Cc = min(128, LAYERS[l][0] - c0)
                    sq = sqp.tile([128, R], BF16, tag="sq", name="sq")
                    sqs.append((sq, Cc))
                    if sq_eng == "act":
                        nc.scalar.square(sq[0:Cc, :], rt[(l, i)][:])
                    else:
                        nc.gpsimd.tensor_tensor(sq[0:Cc, :], rt[(l, i)][:],
                                                rt[(l, i)][:],
                                                op=mybir.AluOpType.mult)
                for ci, (c0c, n) in enumerate(
                        [(0, 512), (512, 512), (1024, 512), (1536, 464)]):
                    rn2ps = pR.tile([1, 512], F32, tag="rn2ps", name="rn2ps")
                    for i, (sq, Cc) in enumerate(sqs):
                        nc.tensor.matmul(rn2ps[0:1, 0:n],
                                         onesb[0:Cc, 0:1],
                                         sq[0:Cc, c0c:c0c + n],
                                         start=(i == 0),
                                         stop=(i == len(sqs) - 1))
                    if ci % 2 == 0:
                        nc.scalar.copy(rn2sb[0:1, l * R + c0c:l * R + c0c + n],
                                       rn2ps[0:1, 0:n])
                    else:
                        nc.vector.tensor_copy(
                            rn2sb[0:1, l * R + c0c:l * R + c0c + n],
                            rn2ps[0:1, 0:n])
            nc.sync.dma_start(out=rc_all[1:2, :], in_=rn2sb[:])

            # ---------------- per-layer compute pipelines
            # qTe also hosts qn2 (cols 240+QCC[l], partition 0) and the
            # w-broadcast psum (cols 368:373) to stay within one PSUM bank
            qTe = pQ.tile([128, 373], F32, tag="qTe", name="qTe")
            qTs = pp.tile([128, 224], FP8, tag="qTs", name="qTs")
            qTs0c = pp.tile([64, 32], FP8, tag="qTs0c", name="qTs0c")
            qsq = pp.tile([128, 128], BF16, tag="qsq", name="qsq")
            tbA = pp.tile([128, 1000], F32, tag="tbA", name="tbA")
            tbB = pp.tile([128, 1000], F32, tag="tbB", name="tbB")

            def pool_l0():
                for t in range(3):
                    for p in range(16):
                        for h in range(8):
                            nc.tensor.matmul(qTe[:, 224 + p:225 + p],
                                             f0[t][:, p, h, :],
                                             ones8[:, 0:1],
                                             start=(t == 0 and h == 0),
                                             stop=False)
                for p in range(16):
                    for h in range(8):
                        nc.tensor.matmul(qTe[:, 224 + p:225 + p],
                                         f0t[0:8, p, h, :],
                                         ones8[0:8, 0:1],
                                         start=False, stop=(h == 7))

            def pool_l3():
                for j in range(B):
                    s = int(PERM[j])
                    for a in range(4):
                        nc.tensor.matmul(qTe[:, 32 * a + j:32 * a + j + 1],
                                         f3[:, s, 128 * a:128 * (a + 1)],
                                         ones8[0:49, 0:1],
                                         start=True, stop=True)

            def pool_l2():
                for j in range(B):
                    s = int(PERM[j])
                    for a in range(2):
                        for h in range(2):
                            nc.tensor.matmul(
                                qTe[:, 128 + 32 * a + j:128 + 32 * a + j + 1],
                                f2[:, s, h, 128 * a:128 * (a + 1)],
                                ones8[0:98, 0:1],
                                start=(h == 0), stop=(h == 1))

            def pool_l1():
                # all f1a matmuls first: the f1b DMA lands last and must not
                # block the in-order PE queue
                for j in range(B):
                    s = int(PERM[j])
                    for h in range(4):
                        nc.tensor.matmul(qTe[:, 192 + j:192 + j + 1],
                                         f1a[:, s, h, :], ones8[:, 0:1],
                                         start=(h == 0), stop=False)
                for j in range(B):
                    s = int(PERM[j])
                    for h in range(4):
                        nc.tensor.matmul(qTe[:, 192 + j:192 + j + 1],
                                         f1b[:, s, h, :], ones8[0:68, 0:1],
                                         start=False, stop=(h == 3))

            def qphase(l):
                """evict scaled q, compute -qn2 into qcx; layer ready for
                distance matmuls afterwards"""
                C, HW = LAYERS[l]
                qb = QCOL[l]
                if l == 0:
                    nc.scalar.activation(qTs0c[:, 0:16], qTe[0:64, 224:240],
                                         mybir.ActivationFunctionType.Copy,
                                         scale=2.0 / HW)
                    nc.scalar.activation(qTs0c[:, 16:32], qTe[64:128, 224:240],
                                         mybir.ActivationFunctionType.Copy,
                                         scale=2.0 / HW)
                    nc.scalar.activation(qsq[0:64, 96:128], qTs0c[:],
                                         mybir.ActivationFunctionType.Square,
                                         scale=0.5)
                    nc.tensor.matmul(qTe[0:1, 336:368], onesb[0:64, 0:1],
                                     qsq[0:64, 96:128], start=True, stop=True)
                    nc.scalar.activation(qcx[0:1, 96:128],
                                         qTe[0:1, 336:368],
                                         mybir.ActivationFunctionType.Copy,
                                         scale=-1.0)
                    return
                nch = C // 128
                nc.scalar.activation(qTs[:, qb:qb + 32 * nch],
                                     qTe[:, qb:qb + 32 * nch],
                                     mybir.ActivationFunctionType.Copy,
                                     scale=2.0 / HW)
                nc.scalar.activation(qsq[:, 0:32 * nch], qTe[:, qb:qb + 32 * nch],
                                     mybir.ActivationFunctionType.Square,
                                     scale=1.0 / HW)
                for a in range(nch):
                    nc.tensor.matmul(qTe[0:1, 240 + QCC[l]:240 + QCC[l] + 32],
                                     onesb[:, 0:1],
                                     qsq[:, 32 * a:32 * a + 32],
                                     start=(a == 0), stop=(a == nch - 1))
                nc.scalar.activation(qcx[0:1, QCC[l]:QCC[l] + 32],
                                     qTe[0:1, 240 + QCC[l]:240 + QCC[l] + 32],
                                     mybir.ActivationFunctionType.Copy,
                                     scale=-1.0)

            def dist(l, hi, evict):
                h0, subs = HALves[hi]
                dps = pD.tile([B, 1000], F32, tag="dps", name="dps")
                for si, (c0, n) in enumerate(subs):
                    d = c0 - h0
                    if l == 0:
                        nc.tensor.matmul(dps[:, d:d + n], qTs0c[:],
                                         rt[(0, 0)][:, c0:c0 + n],
                                         start=True, stop=False)
                    else:
                        qb = QCOL[l]
                        for i, _c0 in enumerate(rt_chunks[l]):
                            nc.tensor.matmul(
                                dps[:, d:d + n],
                                qTs[:, qb + 32 * i:qb + 32 * i + 32],
                                rt[(l, i)][:, c0:c0 + n],
                                start=(i == 0), stop=False)
                    nc.tensor.matmul(dps[:, d:d + n],
                                     qcx[:, QCC[l]:QCC[l] + 32],
                                     rc_all[:, l * R + c0:l * R + c0 + n],
                                     start=False, stop=True)
                dst = (tbA if hi == 0 else tbB)[32 * l:32 * l + 32, :]
                if evict == "act":
                    nc.scalar.copy(dst, dps[:])
                else:
                    nc.vector.tensor_copy(dst, dps[:])

            # layer pipelines in data-arrival order; L1 last
            pool_l0()
            qphase(0)
            dist(0, 0, "dve")
            dist(0, 1, "act")
            pool_l3()
            qphase(3)
            dist(3, 0, "act")
            dist(3, 1, "dve")
            pool_l2()
            qphase(2)
            dist(2, 0, "dve")
            dist(2, 1, "act")
            pool_l1()
            qphase(1)

            vals = pp.tile([128, 48], F32, tag="vals", name="vals")
            v24 = pp.tile([128, 24], F32, tag="v24", name="v24")

            def topk_half(tb, dstcol):
                nc.vector.max(vals[:, dstcol:dstcol + 8], tb[:])
                nc.vector.match_replace(tb[:], vals[:, dstcol:dstcol + 8],
                                        tb[:], NEG_BIG)
                nc.vector.max(vals[:, dstcol + 8:dstcol + 16], tb[:])
                nc.vector.match_replace(tb[:], vals[:, dstcol + 8:dstcol + 16],
                                        tb[:], NEG_BIG)
                nc.vector.max(vals[:, dstcol + 16:dstcol + 24], tb[:])

            dist(1, 0, "act")
            topk_half(tbA, 0)
            dist(1, 1, "act")
            topk_half(tbB, 24)

            # merge 48 -> 24
            nc.vector.max(v24[:, 0:8], vals[:])
            nc.vector.match_replace(vals[:], v24[:, 0:8], vals[:], NEG_BIG)
            nc.vector.max(v24[:, 8:16], vals[:])
            nc.vector.match_replace(vals[:], v24[:, 8:16], vals[:], NEG_BIG)
            nc.vector.max(v24[:, 16:24], vals[:])

            # ---------------- LID
            ln2 = pp.tile([128, 24], F32, tag="ln2", name="ln2")
            S = pp.tile([128, 1], F32, tag="S", name="S")
            denom = pp.tile([128, 1], F32, tag="denom", name="denom")
            lid = pp.tile([128, 1], F32, tag="lid", name="lid")
            nc.scalar.activation(ln2[:], v24[:],
                                 mybir.ActivationFunctionType.Ln, scale=-1.0)
            nc.vector.tensor_reduce(S[:], ln2[:, 1:21],
                                    axis=mybir.AxisListType.X,
                                    op=mybir.AluOpType.add)
            nc.vector.tensor_scalar(denom[:], ln2[:, 20:21], -20.0, S[:],
                                    op0=mybir.AluOpType.mult,
                                    op1=mybir.AluOpType.add)
            # lid holds 1/denom; the -2K factor is folded into the host-side
            # regression weights
            nc.vector.reciprocal(lid[:], denom[:])

            # ---------------- regression + sigmoid(x) = 1/(1+exp(-x))
            lid4 = pp.tile([B, 4], F32, tag="lid4", name="lid4")
            for l in range(4):
                nc.vector.tensor_copy(lid4[:, l:l + 1],
                                      lid[32 * l:32 * l + 32, :])
            nc.tensor.matmul(qTe[0:B, 368:373], ones_row[:], wb_sb[:],
                             start=True, stop=True)
            wbc = pp.tile([B, 5], F32, tag="wbc", name="wbc")
            nc.scalar.copy(wbc[:], qTe[0:B, 368:373])
            prod = pp.tile([B, 4], F32, tag="prod", name="prod")
            nc.vector.tensor_tensor(prod[:], lid4[:], wbc[:, 0:4],
                                    op=mybir.AluOpType.mult)
            ssum = pp.tile([B, 1], F32, tag="ssum", name="ssum")
            nc.vector.tensor_reduce(ssum[:], prod[:],
                                    axis=mybir.AxisListType.X,
                                    op=mybir.AluOpType.add)
            # logit = ssum + b;  res = 1/(1 + exp(-logit))
            enx = pp.tile([B, 1], F32, tag="enx", name="enx")
            nc.vector.tensor_tensor(enx[:], ssum[:], wbc[:, 4:5],
                                    op=mybir.AluOpType.add)
            nc.scalar.activation(enx[:], enx[:],
                                 mybir.ActivationFunctionType.Exp, scale=-1.0)
            res = pp.tile([B, 1], F32, tag="res", name="res")
            nc.vector.tensor_scalar(res[:], enx[:], 1.0, None,
                                    op0=mybir.AluOpType.add)
            nc.vector.reciprocal(res[:], res[:])
            nc.sync.dma_start(out=out[:], in_=res[:])

    nc.compile()
    return nc


_NC = None


def _get_nc():
    global _NC
    if _NC is None:
        _NC = build_nc()
    return _NC


def run(trace=False, **inputs):
    nc = _get_nc()
    assert int(inputs.get("k", K)) == K

    # host prep: transpose to [B_full, HW, C] fp8 (layer 0 pair-interleaved
    # to [B_full/2, HW, 2*C])
    featsT = []
    for l, (C, HW) in enumerate(LAYERS):
        f = np.asarray(inputs[f"feat{l}"], dtype=np.float32)
        if l == 0:
            f = f.reshape(f.shape[0] // 2, 2, C, HW).transpose(0, 3, 1, 2)
            f = f.reshape(f.shape[0], HW, 2 * C)
        else:
            f = f.reshape(f.shape[0], C, HW).transpose(0, 2, 1)
        featsT.append(np.ascontiguousarray(f).astype(NP_FP8))
    rtsT = [np.ascontiguousarray(
        np.asarray(inputs[f"ref{l}"], dtype=np.float32).T).astype(NP_FP8)
        for l in range(4)]
    regw = np.asarray(inputs["reg_w"], dtype=np.float32).reshape(1, 4)
    regb = np.asarray(inputs["reg_b"], dtype=np.float32).reshape(1, 1)
    regwb = np.concatenate([regw * (-2.0 * K), regb], axis=1)

    in_maps = []
    for c in range(N_CORES):
        m = {f"feat{l}": featsT[l][c * B:(c + 1) * B] for l in range(1, 4)}
        m["feat0"] = featsT[0][c * (B // 2):(c + 1) * (B // 2)]
        for l in range(4):
            m[f"rt{l}"] = rtsT[l]
        m["regwb"] = regwb
        in_maps.append(m)

    res = run_bass_kernel_spmd(nc, in_maps, core_ids=list(range(N_CORES)),
                               trace=trace)
    full = np.empty((N_CORES * B,), dtype=np.float32)
    for c in range(N_CORES):
        shard = np.empty((B,), dtype=np.float32)
        shard[PERM] = res.results[c]["out"][:, 0]
        full[c * B:(c + 1) * B] = shard
    return full, res


def kernel(**inputs):
    return run(trace=False, **inputs)[0]


# revision 49
# speedup vs baseline: 1.0623x; 1.0623x over previous
"""LID detector kernel for Trainium2 (8 NeuronCores, data-parallel over batch).

Per core (batch shard of 32 samples):
  - features arrive host-transposed [B, HW, C] in fp8-e4m3; spatial mean
    pooling runs on the TensorEngine as ones-vector matmuls (reduction over
    the partition axis = hw), accumulating q directly in [C, B] layout in
    PSUM.  Layer 0 is pair-interleaved ([16, HW, 2*C]) so each matmul pools
    two samples.
  - reference tables arrive host-transposed [C, R] fp8; rn2 = ||r||^2 via
    square (ACT/gpsimd) + ones matmul.
  - -d2 = 2q.r - rn2 - qn2 accumulated fully in PSUM: C-chunk matmuls plus
    one K=2 matmul with lhsT [[-qn2],[-1]] and rhs [[ones],[rn2]]; eviction
    is then a plain copy into the topk buffer.
  - per-layer pipelines: each layer's pooling, qn2, and distance matmuls run
    as soon as its feature DMA lands; layer 1 is loaded last (cheapest tail).
  - top-24 smallest d2 via DVE max8 + match_replace on two column halves,
    then a 48-wide merge; LID = -2k / (sum_{i=2..21} ln d2_i - 20 ln d2_21).
  - logit = w . lid + b; sigmoid computed as 1/(1+exp(-logit)) so Ln and Exp
    share one ACT table set (preloaded by a dummy Ln at kernel start).
Sample order inside a core is PERM (evens then odds, from the layer-0 pair
packing); the host inverts it on gather.
"""

import sys

for _p in ("/opt/trn_rl_repo", "/root/.axon_site/_ro/trn_rl_repo"):
    if _p not in sys.path:
        sys.path.append(_p)

import ml_dtypes
import numpy as np

import concourse.mybir as mybir
from concourse import bass, bacc
from concourse.tile import TileContext
from concourse.bass_utils import run_bass_kernel_spmd

F32 = mybir.dt.float32
BF16 = mybir.dt.bfloat16
FP8 = mybir.dt.float8e4
NP_FP8 = ml_dtypes.float8_e4m3

N_CORES = 8
B = 32          # batch shard per core
R = 2000
K = 20
NEG_BIG = -3.0e38
LAYERS = [(64, 3136), (128, 784), (256, 196), (512, 49)]  # (C, H*W)

# column j of the on-device layout holds sample PERM[j] of the local shard
PERM = np.array([2 * j for j in range(16)] + [2 * j + 1 for j in range(16)])

# qTe column base per layer (layer 0 packed 2-per-column at 224:240)
QCOL = {3: 0, 2: 128, 1: 192, 0: 224}
# qc lhsT column base per layer within qcx [2, 128]
QCC = {3: 0, 2: 32, 1: 64, 0: 96}
# distance column halves, each split into <=512-wide matmul sub-chunks
HALves = [(0, [(0, 512), (512, 488)]), (1000, [(1000, 512), (1512, 488)])]


PHASES = []


def build_nc():
    nc = bacc.Bacc("TRN2", target_bir_lowering=False, debug=False,
                   num_devices=N_CORES)
    del PHASES[:]

    def _mark(label):
        # burn one instruction name to record the emission cursor
        n = int(nc.get_next_instruction_name().split("-")[1])
        PHASES.append((n, label))

    feats = [nc.dram_tensor(
        "feat0" if l == 0 else f"feat{l}",
        [B // 2, HW, 2 * C] if l == 0 else [B, HW, C],
        FP8, kind="ExternalInput") for l, (C, HW) in enumerate(LAYERS)]
    rts = [nc.dram_tensor(f"rt{l}", [C, R], FP8, kind="ExternalInput")
           for l, (C, _) in enumerate(LAYERS)]
    # regwb = [-2K * w, b]  (folded on the host)
    regwb = nc.dram_tensor("regwb", [1, 5], F32, kind="ExternalInput")
    out = nc.dram_tensor("out", [B, 1], F32, kind="ExternalOutput")

    rt_chunks = {l: list(range(0, C, 128)) for l, (C, _) in enumerate(LAYERS)}

    with TileContext(nc) as tc:
        with (
            tc.tile_pool(name="pp", bufs=1) as pp,
            tc.tile_pool(name="sq", bufs=4) as sqp,
            tc.tile_pool(name="pQ", bufs=1, space=bass.MemorySpace.PSUM) as pQ,
        ):
            # ---------------- persistent SBUF tiles + constants
            ones8 = pp.tile([128, 1], FP8, tag="ones8", name="ones8")
            onesb = pp.tile([128, 1], BF16, tag="onesb", name="onesb")
            ones_row = pp.tile([1, B], F32, tag="ones_row", name="ones_row")
            nc.vector.memset(ones8[:], 1.0)
            nc.vector.memset(onesb[:], 1.0)
            nc.vector.memset(ones_row[:], 1.0)
            # preload the natural_log_exp_and_others ACT table set; all ACT
            # ops used below (copy/square/ln/exp) live in this one set, so no
            # further table loads are inserted
            nc.scalar.add_instruction(mybir.InstLoadActFuncSet(
                name=nc.get_next_instruction_name(), ins=[], outs=[],
                act_func_set_id=6))

            wb_sb = pp.tile([1, 5], F32, tag="wb_sb", name="wb_sb")
            nc.sync.dma_start(out=wb_sb[:], in_=regwb[:])

            rt = {}
            for l, (C, _) in enumerate(LAYERS):
                for i, c0 in enumerate(rt_chunks[l]):
                    Cc = min(128, C - c0)
                    rt[(l, i)] = pp.tile([Cc, R], FP8, tag=f"rt{l}_{i}",
                                         name=f"rt{l}_{i}")

            # combo-matmul operands (engines may only write partition 0; the
            # partition-1 rows are filled by small SBUF->SBUF DMAs):
            #   lhsT qcx: row0 = -qn2 per sample, row1 = -1
            #   rhs  rc_all: row0 = ones, row1 = rn2  (cols l*R + c)
            rn2sb = pp.tile([1, 4 * R], BF16, tag="rn2sb", name="rn2sb")
            rc_all = pp.tile([2, 4 * R], BF16, tag="rc_all", name="rc_all")
            qcx = pp.tile([2, 128], BF16, tag="qcx", name="qcx")
            negones = pp.tile([1, 128], BF16, tag="negones", name="negones")
            nc.vector.memset(rc_all[0:1, :], 1.0)
            nc.vector.memset(negones[:], -1.0)
            nc.sync.dma_start(out=qcx[1:2, :], in_=negones[:])

            _mark("dmas")
            # ---------------- DMA issue order = rough schedule
            for l in (0, 3, 2, 1):
                for i, c0 in enumerate(rt_chunks[l]):
                    Cc = min(128, LAYERS[l][0] - c0)
                    nc.sync.dma_start(out=rt[(l, i)][:],
                                      in_=rts[l][c0:c0 + Cc, :])

            C0, HW0 = LAYERS[0]   # 3136 = 3*1024 + 8*8; free = (pair, 8, 2C)
            f0 = [pp.tile([128, 16, 8, 2 * C0], FP8, tag=f"f0_{t}",
                          name=f"f0_{t}") for t in range(3)]
            for t in range(3):
                nc.sync.dma_start(
                    out=f0[t][:],
                    in_=bass.AP(feats[0], t * 1024 * 2 * C0,
                                [[16 * C0, 128], [HW0 * 2 * C0, 16],
                                 [1, 16 * C0]]))
            f0t = pp.tile([8, 16, 8, 2 * C0], FP8, tag="f0t", name="f0t")
            nc.sync.dma_start(
                out=f0t[:], in_=bass.AP(feats[0], 3072 * 2 * C0,
                                        [[16 * C0, 8], [HW0 * 2 * C0, 16],
                                         [1, 16 * C0]]))

            C3, HW3 = LAYERS[3]
            f3 = pp.tile([49, B, C3], FP8, tag="f3", name="f3")
            nc.sync.dma_start(
                out=f3[:], in_=bass.AP(feats[3], 0,
                                       [[C3, 49], [HW3 * C3, B], [1, C3]]))

            C2, HW2 = LAYERS[2]
            f2 = pp.tile([98, B, 2, C2], FP8, tag="f2", name="f2")
            nc.sync.dma_start(
                out=f2[:], in_=bass.AP(feats[2], 0,
                                       [[2 * C2, 98], [HW2 * C2, B],
                                        [1, 2 * C2]]))

            C1, HW1 = LAYERS[1]   # 784 = 128*4 + 68*4
            f1a = pp.tile([128, B, 4, C1], FP8, tag="f1a", name="f1a")
            nc.sync.dma_start(
                out=f1a[:], in_=bass.AP(feats[1], 0,
                                        [[4 * C1, 128], [HW1 * C1, B],
                                         [1, 4 * C1]]))
            f1b = pp.tile([68, B, 4, C1], FP8, tag="f1b", name="f1b")
            nc.sync.dma_start(
                out=f1b[:], in_=bass.AP(feats[1], 512 * C1,
                                        [[4 * C1, 68], [HW1 * C1, B],
                                         [1, 4 * C1]]))

            _mark("rn2")
            # ---------------- rn2 = sum_c r^2 (squares on ACT, ones-matmul
            # reductions into two [1,1000] psum tiles, one evict per half;
            # rc_all row 1 is DMA-filled PER LAYER so each layer's distance
            # combos gate only on that layer's rn2)
            with tc.tile_pool(name="pR", bufs=2,
                              space=bass.MemorySpace.PSUM) as pR:
                for l in (0, 3, 2, 1):
                    chunks = rt_chunks[l]
                    sqs = []
                    for i, c0 in enumerate(chunks):
                        Cc = min(128, LAYERS[l][0] - c0)
                        sq = sqp.tile([128, R], BF16, tag="sq", name="sq")
                        sqs.append((sq, Cc))
                        nc.scalar.square(sq[0:Cc, :], rt[(l, i)][:])
                    for hi, (h0, subs) in enumerate(HALves):
                        rn2ps = pR.tile([1, 1000], F32, tag="rn2ps",
                                        name="rn2ps")
                        for c0c, n in subs:
                            d = c0c - h0
                            for i, (sq, Cc) in enumerate(sqs):
                                nc.tensor.matmul(rn2ps[0:1, d:d + n],
                                                 onesb[0:Cc, 0:1],
                                                 sq[0:Cc, c0c:c0c + n],
                                                 start=(i == 0),
                                                 stop=(i == len(sqs) - 1))
                        if hi == 0:
                            nc.scalar.copy(
                                rn2sb[0:1, l * R + h0:l * R + h0 + 1000],
                                rn2ps[:])
                        else:
                            nc.vector.tensor_copy(
                                rn2sb[0:1, l * R + h0:l * R + h0 + 1000],
                                rn2ps[:])
                    nc.sync.dma_start(
                        out=rc_all[1:2, l * R:(l + 1) * R],
                        in_=rn2sb[0:1, l * R:(l + 1) * R])

            # ---------------- per-layer compute pipelines
            # qTe also hosts qn2 (cols 240+QCC[l], partition 0) and the
            # w-broadcast psum (cols 368:373) to stay within one PSUM bank
            qTe = pQ.tile([128, 373], F32, tag="qTe", name="qTe")
            qTs = pp.tile([128, 224], FP8, tag="qTs", name="qTs")
            qTs0c = pp.tile([64, 32], FP8, tag="qTs0c", name="qTs0c")
            qsq = pp.tile([128, 128], BF16, tag="qsq", name="qsq")
            tbA = pp.tile([128, 1000], F32, tag="tbA", name="tbA")
            tbB = pp.tile([128, 1000], F32, tag="tbB", name="tbB")

            def pool_l0():
                for t in range(3):
                    for p in range(16):
                        for h in range(8):
                            nc.tensor.matmul(qTe[:, 224 + p:225 + p],
                                             f0[t][:, p, h, :],
                                             ones8[:, 0:1],
                                             start=(t == 0 and h == 0),
                                             stop=False)
                for p in range(16):
                    for h in range(8):
                        nc.tensor.matmul(qTe[:, 224 + p:225 + p],
                                         f0t[0:8, p, h, :],
                                         ones8[0:8, 0:1],
                                         start=False, stop=(h == 7))

            def pool_l3():
                for j in range(B):
                    s = int(PERM[j])
                    for a in range(4):
                        nc.tensor.matmul(qTe[:, 32 * a + j:32 * a + j + 1],
                                         f3[:, s, 128 * a:128 * (a + 1)],
                                         ones8[0:49, 0:1],
                                         start=True, stop=True)

            def pool_l2():
                for j in range(B):
                    s = int(PERM[j])
                    for a in range(2):
                        for h in range(2):
                            nc.tensor.matmul(
                                qTe[:, 128 + 32 * a + j:128 + 32 * a + j + 1],
                                f2[:, s, h, 128 * a:128 * (a + 1)],
                                ones8[0:98, 0:1],
                                start=(h == 0), stop=(h == 1))

            def pool_l1():
                # all f1a matmuls first: the f1b DMA lands last and must not
                # block the in-order PE queue
                for j in range(B):
                    s = int(PERM[j])
                    for h in range(4):
                        nc.tensor.matmul(qTe[:, 192 + j:192 + j + 1],
                                         f1a[:, s, h, :], ones8[:, 0:1],
                                         start=(h == 0), stop=False)
                for j in range(B):
                    s = int(PERM[j])
                    for h in range(4):
                        nc.tensor.matmul(qTe[:, 192 + j:192 + j + 1],
                                         f1b[:, s, h, :], ones8[0:68, 0:1],
                                         start=False, stop=(h == 3))

            def qphase(l):
                """evict scaled q, compute -qn2 into qcx; layer ready for
                distance matmuls afterwards"""
                C, HW = LAYERS[l]
                qb = QCOL[l]
                if l == 0:
                    nc.scalar.activation(qTs0c[:, 0:16], qTe[0:64, 224:240],
                                         mybir.ActivationFunctionType.Copy,
                                         scale=2.0 / HW)
                    nc.scalar.activation(qTs0c[:, 16:32], qTe[64:128, 224:240],
                                         mybir.ActivationFunctionType.Copy,
                                         scale=2.0 / HW)
                    nc.scalar.activation(qsq[0:64, 96:128], qTs0c[:],
                                         mybir.ActivationFunctionType.Square,
                                         scale=0.5)
                    nc.tensor.matmul(qTe[0:1, 336:368], onesb[0:64, 0:1],
                                     qsq[0:64, 96:128], start=True, stop=True)
                    nc.scalar.activation(qcx[0:1, 96:128],
                                         qTe[0:1, 336:368],
                                         mybir.ActivationFunctionType.Copy,
                                         scale=-1.0)
                    return
                nch = C // 128
                nc.scalar.activation(qTs[:, qb:qb + 32 * nch],
                                     qTe[:, qb:qb + 32 * nch],
                                     mybir.ActivationFunctionType.Copy,
                                     scale=2.0 / HW)
                nc.scalar.activation(qsq[:, 0:32 * nch], qTe[:, qb:qb + 32 * nch],
                                     mybir.ActivationFunctionType.Square,
                                     scale=1.0 / HW)
                for a in range(nch):
                    nc.tensor.matmul(qTe[0:1, 240 + QCC[l]:240 + QCC[l] + 32],
                                     onesb[:, 0:1],
                                     qsq[:, 32 * a:32 * a + 32],
                                     start=(a == 0), stop=(a == nch - 1))
                nc.scalar.activation(qcx[0:1, QCC[l]:QCC[l] + 32],
                                     qTe[0:1, 240 + QCC[l]:240 + QCC[l] + 32],
                                     mybir.ActivationFunctionType.Copy,
                                     scale=-1.0)

            def dist(l, hi, evict):
                h0, subs = HALves[hi]
                dps = pD.tile([B, 1000], F32, tag="dps", name="dps")
                for si, (c0, n) in enumerate(subs):
                    d = c0 - h0
                    if l == 0:
                        nc.tensor.matmul(dps[:, d:d + n], qTs0c[:],
                                         rt[(0, 0)][:, c0:c0 + n],
                                         start=True, stop=False)
                    else:
                        qb = QCOL[l]
                        for i, _c0 in enumerate(rt_chunks[l]):
                            nc.tensor.matmul(
                                dps[:, d:d + n],
                                qTs[:, qb + 32 * i:qb + 32 * i + 32],
                                rt[(l, i)][:, c0:c0 + n],
                                start=(i == 0), stop=False)
                    nc.tensor.matmul(dps[:, d:d + n],
                                     qcx[:, QCC[l]:QCC[l] + 32],
                                     rc_all[:, l * R + c0:l * R + c0 + n],
                                     start=False, stop=True)
                dst = (tbA if hi == 0 else tbB)[32 * l:32 * l + 32, :]
                if evict == "act":
                    nc.scalar.copy(dst, dps[:])
                else:
                    nc.vector.tensor_copy(dst, dps[:])

            _mark("pipe")
            # layer pipelines in data-arrival order; L1 last
            pD_cm = tc.tile_pool(name="pD", bufs=3,
                                 space=bass.MemorySpace.PSUM)
            pD = pD_cm.__enter__()
            _mark("pool0")
            pool_l0()
            _mark("q0")
            qphase(0)
            _mark("dist0")
            dist(0, 0, "dve")
            dist(0, 1, "act")
            _mark("pool3")
            pool_l3()
            _mark("q3")
            qphase(3)
            _mark("dist3")
            dist(3, 0, "act")
            dist(3, 1, "dve")
            _mark("pool2")
            pool_l2()
            _mark("q2")
            qphase(2)
            _mark("dist2")
            dist(2, 0, "dve")
            dist(2, 1, "act")
            _mark("pool1")
            pool_l1()
            _mark("q1")
            qphase(1)

            vals = pp.tile([128, 48], F32, tag="vals", name="vals")
            v24 = pp.tile([128, 24], F32, tag="v24", name="v24")

            def topk_half(tb, dstcol):
                nc.vector.max(vals[:, dstcol:dstcol + 8], tb[:])
                nc.vector.match_replace(tb[:], vals[:, dstcol:dstcol + 8],
                                        tb[:], NEG_BIG)
                nc.vector.max(vals[:, dstcol + 8:dstcol + 16], tb[:])
                nc.vector.match_replace(tb[:], vals[:, dstcol + 8:dstcol + 16],
                                        tb[:], NEG_BIG)
                nc.vector.max(vals[:, dstcol + 16:dstcol + 24], tb[:])

            _mark("dist1A")
            dist(1, 0, "act")
            _mark("topkA")
            topk_half(tbA, 0)
            _mark("dist1B")
            dist(1, 1, "act")
            _mark("topkB")
            topk_half(tbB, 24)

            _mark("merge")
            # merge 48 -> 24
            nc.vector.max(v24[:, 0:8], vals[:])
            nc.vector.match_replace(vals[:], v24[:, 0:8], vals[:], NEG_BIG)
            nc.vector.max(v24[:, 8:16], vals[:])
            nc.vector.match_replace(vals[:], v24[:, 8:16], vals[:], NEG_BIG)
            nc.vector.max(v24[:, 16:24], vals[:])

            _mark("lid")
            # ---------------- LID
            ln2 = pp.tile([128, 24], F32, tag="ln2", name="ln2")
            S = pp.tile([128, 1], F32, tag="S", name="S")
            denom = pp.tile([128, 1], F32, tag="denom", name="denom")
            lid = pp.tile([128, 1], F32, tag="lid", name="lid")
            nc.scalar.activation(ln2[:], v24[:],
                                 mybir.ActivationFunctionType.Ln, scale=-1.0)
            nc.vector.tensor_reduce(S[:], ln2[:, 1:21],
                                    axis=mybir.AxisListType.X,
                                    op=mybir.AluOpType.add)
            nc.vector.tensor_scalar(denom[:], ln2[:, 20:21], -20.0, S[:],
                                    op0=mybir.AluOpType.mult,
                                    op1=mybir.AluOpType.add)
            # lid holds 1/denom; the -2K factor is folded into the host-side
            # regression weights
            nc.vector.reciprocal(lid[:], denom[:])

            _mark("reg")
            # ---------------- regression + sigmoid(x) = 1/(1+exp(-x))
            lid4 = pp.tile([B, 4], F32, tag="lid4", name="lid4")
            for l in range(4):
                nc.vector.tensor_copy(lid4[:, l:l + 1],
                                      lid[32 * l:32 * l + 32, :])
            nc.tensor.matmul(qTe[0:B, 368:373], ones_row[:], wb_sb[:],
                             start=True, stop=True)
            wbc = pp.tile([B, 5], F32, tag="wbc", name="wbc")
            nc.scalar.copy(wbc[:], qTe[0:B, 368:373])
            prod = pp.tile([B, 4], F32, tag="prod", name="prod")
            nc.vector.tensor_tensor(prod[:], lid4[:], wbc[:, 0:4],
                                    op=mybir.AluOpType.mult)
            ssum = pp.tile([B, 1], F32, tag="ssum", name="ssum")
            nc.vector.tensor_reduce(ssum[:], prod[:],
                                    axis=mybir.AxisListType.X,
                                    op=mybir.AluOpType.add)
            # logit = ssum + b;  res = 1/(1 + exp(-logit))
            enx = pp.tile([B, 1], F32, tag="enx", name="enx")
            nc.vector.tensor_tensor(enx[:], ssum[:], wbc[:, 4:5],
                                    op=mybir.AluOpType.add)
            nc.scalar.activation(enx[:], enx[:],
                                 mybir.ActivationFunctionType.Exp, scale=-1.0)
            res = pp.tile([B, 1], F32, tag="res", name="res")
            nc.vector.tensor_scalar(res[:], enx[:], 1.0, None,
                                    op0=mybir.AluOpType.add)
            nc.vector.reciprocal(res[:], res[:])
            nc.sync.dma_start(out=out[:], in_=res[:])
            pD_cm.__exit__(None, None, None)

    nc.compile()
    return nc


_NC = None


def _get_nc():
    global _NC
    if _NC is None:
        _NC = build_nc()
    return _NC


def run(trace=False, **inputs):
    nc = _get_nc()
    assert int(inputs.get("k", K)) == K

    # host prep: transpose to [B_full, HW, C] fp8 (layer 0 pair-interleaved
    # to [B_full/2, HW, 2*C])
    featsT = []
    for l, (C, HW) in enumerate(LAYERS):
        f = np.asarray(inputs[f"feat{l}"], dtype=np.float32)
        if l == 0:
            f = f.reshape(f.shape[0] // 2, 2, C, HW).transpose(0, 3, 1, 2)
            f = f.reshape(f.shape[0], HW, 2 * C)
        else:
            f = f.reshape(f.shape[0], C, HW).transpose(0, 2, 1)
        featsT.append(np.ascontiguousarray(f).astype(NP_FP8))
    rtsT = [np.ascontiguousarray(
        np.asarray(inputs[f"ref{l}"], dtype=np.float32).T).astype(NP_FP8)
        for l in range(4)]
    regw = np.asarray(inputs["reg_w"], dtype=np.float32).reshape(1, 4)
    regb = np.asarray(inputs["reg_b"], dtype=np.float32).reshape(1, 1)
    regwb = np.concatenate([regw * (-2.0 * K), regb], axis=1)

    in_maps = []
    for c in range(N_CORES):
        m = {f"feat{l}": featsT[l][c * B:(c + 1) * B] for l in range(1, 4)}
        m["feat0"] = featsT[0][c * (B // 2):(c + 1) * (B // 2)]
        for l in range(4):
            m[f"rt{l}"] = rtsT[l]
        m["regwb"] = regwb
        in_maps.append(m)

    res = run_bass_kernel_spmd(nc, in_maps, core_ids=list(range(N_CORES)),
                               trace=trace)
    full = np.empty((N_CORES * B,), dtype=np.float32)
    for c in range(N_CORES):
        shard = np.empty((B,), dtype=np.float32)
        shard[PERM] = res.results[c]["out"][:, 0]
        full[c * B:(c + 1) * B] = shard
    return full, res


def kernel(**inputs):
    return run(trace=False, **inputs)[0]
